# revision 11
# baseline (speedup 1.0000x reference)
"""MoE FFN (8 routed experts top-2 + 1 shared expert) on 8 TRN2 NeuronCores.

Sharding: expert-parallel. Core c holds routed expert c's weights and computes
that expert densely for all 4096 tokens, scaled by the (renormalized top-2)
combine weight for expert c (zero for tokens that didn't pick it). Core c also
computes the shared expert for its 512-token slice. A ReduceScatter over the
per-core partials [4096, 1024] sums expert contributions and hands core c the
token slice [512c:512(c+1)); the shared-expert slice is added locally and each
core emits its 512-token output shard, concatenated on the host.

All matmuls run as float32r (full-rate fp32 on the PE when the moving dim is
>= 256), accumulating in fp32 PSUM.
"""

import numpy as np

import concourse.bacc as bacc
import concourse.bass as bass
import concourse.mybir as mybir
import concourse.tile as tile
from concourse.bass_utils import run_bass_kernel_spmd

P = 128
C = 1024          # d_model
H = 2048          # d_expert
T = 4096          # tokens (2*2048)
E = 8             # routed experts = cores
TOPK = 2
TS = T // E       # 512 tokens per core slice
TB = 256          # token block for the routed phases (moving dim >= 256)
CC = C // P       # 8 c-chunks
HC = H // P       # 16 h-chunks
F32 = mybir.dt.float32
F32R = mybir.dt.float32r

NCORES = 8

# Hardware has a native Silu; the simulator doesn't. Flip to False for sim runs.
SILU_NATIVE = True


def _silu_mul(nc, act_sb, up_ps, gt_ps):
    if SILU_NATIVE:
        nc.scalar.activation(act_sb, up_ps,
                             mybir.ActivationFunctionType.Silu)
    else:
        nc.scalar.activation(act_sb, up_ps,
                             mybir.ActivationFunctionType.Sigmoid)
        nc.vector.tensor_mul(act_sb, act_sb, up_ps)
    nc.vector.tensor_mul(act_sb, act_sb, gt_ps)


def _build_program(T=T, C=C, H=H, TS=TS, TB=TB):
    CC = C // P
    HC = H // P
    nc = bacc.Bacc("TRN2", target_bir_lowering=False, debug=False,
                   num_devices=NCORES)

    # ---- per-core inputs ----
    x = nc.dram_tensor("x", [T, C], F32R, kind="ExternalInput")       # replicated
    xs = nc.dram_tensor("xs", [TS, C], F32R, kind="ExternalInput")    # token slice
    rwu = nc.dram_tensor("rwu", [C, H], F32R, kind="ExternalInput")   # expert up
    rwg = nc.dram_tensor("rwg", [C, H], F32R, kind="ExternalInput")   # expert gate
    rwd = nc.dram_tensor("rwd", [H, C], F32R, kind="ExternalInput")   # expert down
    swu = nc.dram_tensor("swu", [C, H], F32R, kind="ExternalInput")
    swg = nc.dram_tensor("swg", [C, H], F32R, kind="ExternalInput")
    swd = nc.dram_tensor("swd", [H, C], F32R, kind="ExternalInput")
    rtw = nc.dram_tensor("rtw", [C, E], F32, kind="ExternalInput")   # router
    ohx = nc.dram_tensor("ohx", [P, E], F32, kind="ExternalInput")    # bcast 1hot
    idn = nc.dram_tensor("idn", [P, P], F32R, kind="ExternalInput")   # identity

    out = nc.dram_tensor("out", [TS, C], F32, kind="ExternalOutput")

    # ---- internal DRAM ----
    acts_s = nc.dram_tensor("acts_s", [H, TS], F32R)       # shared-expert act spill
    acts_r = nc.dram_tensor("acts_r", [H, T], F32R)        # routed act spill
    partial = nc.dram_tensor("partial", [T, C], F32)       # pre-reduce partial
    rs_out = nc.dram_tensor("rs_out", [TS, C], F32)

    with tile.TileContext(nc) as tc:
        with tc.tile_pool(name="persist", bufs=1) as pp:
            ident = pp.tile([P, P], F32R)
            nc.sync.dma_start(ident[:], idn[:])
            ohb = pp.tile([P, E], F32)
            nc.sync.dma_start(ohb[:], ohx[:])

            # per-token combine weight for this core's expert, column j = t-tile
            wv = pp.tile([P, T // P], F32)

            # shared-expert output, kept until after the ReduceScatter
            ys = [pp.tile([P, C], F32, tag=f"ys{i}", name=f"ys{i}") for i in range(TS // P)]

            # ============ shared expert, phase A: up/gate/act ============
            with (
                tc.tile_pool(name="sA", bufs=1) as sa,
                tc.tile_pool(name="sA2", bufs=2) as sa2,
                tc.tile_pool(name="psA", bufs=2, space="PSUM") as psa,
            ):
                # transpose xs -> xst [C, TS] in SBUF
                xst = [sa.tile([P, TS], F32R, tag=f"xst{cc}", name=f"xst{cc}") for cc in range(CC)]
                for ts in range(TS // P):
                    xrow = sa2.tile([P, C], F32R, tag="xrow")
                    nc.sync.dma_start(xrow[:], xs[ts * P:(ts + 1) * P, :])
                    for cc in range(CC):
                        tp = psa.tile([P, P], F32R, tag="tp")
                        nc.tensor.transpose(tp[:], xrow[:, cc * P:(cc + 1) * P],
                                            ident[:])
                        nc.vector.tensor_copy(xst[cc][:, ts * P:(ts + 1) * P], tp[:])

                wu = [sa.tile([P, H], F32R, tag=f"swu{cc}", name=f"swu{cc}") for cc in range(CC)]
                wg = [sa.tile([P, H], F32R, tag=f"swg{cc}", name=f"swg{cc}") for cc in range(CC)]
                for cc in range(CC):
                    nc.sync.dma_start(wu[cc][:], swu[cc * P:(cc + 1) * P, :])
                    nc.sync.dma_start(wg[cc][:], swg[cc * P:(cc + 1) * P, :])

                for hc in range(HC):
                    up_ps = psa.tile([P, TS], F32, tag="up")
                    gt_ps = psa.tile([P, TS], F32, tag="gt")
                    for cc in range(CC):
                        nc.tensor.matmul(up_ps[:], wu[cc][:, hc * P:(hc + 1) * P],
                                         xst[cc][:], start=(cc == 0),
                                         stop=(cc == CC - 1))
                    for cc in range(CC):
                        nc.tensor.matmul(gt_ps[:], wg[cc][:, hc * P:(hc + 1) * P],
                                         xst[cc][:], start=(cc == 0),
                                         stop=(cc == CC - 1))
                    act_sb = sa2.tile([P, TS], F32R, tag="act")
                    _silu_mul(nc, act_sb[:], up_ps[:], gt_ps[:])
                    nc.sync.dma_start(acts_s[hc * P:(hc + 1) * P, :], act_sb[:])

            # ============ shared expert, phase B: down ============
            with (
                tc.tile_pool(name="sB", bufs=1) as sb,
                tc.tile_pool(name="psB", bufs=2, space="PSUM") as psb,
            ):
                wd = [sb.tile([P, C], F32R, tag=f"swd{hc}", name=f"swd{hc}") for hc in range(HC)]
                for hc in range(HC):
                    nc.sync.dma_start(wd[hc][:], swd[hc * P:(hc + 1) * P, :])
                acts = [sb.tile([P, TS], F32R, tag=f"as{hc}", name=f"as{hc}") for hc in range(HC)]
                for hc in range(HC):
                    nc.sync.dma_start(acts[hc][:], acts_s[hc * P:(hc + 1) * P, :])
                for ts in range(TS // P):
                    for cb in range(C // 512):
                        y_ps = psb.tile([P, 512], F32, tag="y")
                        for hc in range(HC):
                            nc.tensor.matmul(
                                y_ps[:],
                                acts[hc][:, ts * P:(ts + 1) * P],
                                wd[hc][:, cb * 512:(cb + 1) * 512],
                                start=(hc == 0), stop=(hc == HC - 1))
                        nc.vector.tensor_copy(ys[ts][:, cb * 512:(cb + 1) * 512],
                                              y_ps[:])

            # ============ routed expert, phase A: router + up/gate/act ==========
            with (
                tc.tile_pool(name="rA", bufs=1) as ra,
                tc.tile_pool(name="rA2", bufs=2) as ra2,
                tc.tile_pool(name="psRA", bufs=1, space="PSUM") as psra,
            ):
                wu = [ra.tile([P, H], F32R, tag=f"rwu{cc}", name=f"rwu{cc}") for cc in range(CC)]
                wg = [ra.tile([P, H], F32R, tag=f"rwg{cc}", name=f"rwg{cc}") for cc in range(CC)]
                rt = [ra.tile([P, E], F32, tag=f"rt{cc}", name=f"rt{cc}") for cc in range(CC)]
                for cc in range(CC):
                    nc.sync.dma_start(wu[cc][:], rwu[cc * P:(cc + 1) * P, :])
                    nc.sync.dma_start(wg[cc][:], rwg[cc * P:(cc + 1) * P, :])
                    nc.sync.dma_start(rt[cc][:], rtw[cc * P:(cc + 1) * P, :])

                for tb in range(T // TB):
                    nsub = TB // P  # 2
                    # load x rows and transpose into xt [C, TB]
                    xt = [ra2.tile([P, TB], F32R, tag=f"xt{cc}", name=f"xt{cc}") for cc in range(CC)]
                    for sub in range(nsub):
                        xrow = ra2.tile([P, C], F32R, tag="xrow")
                        nc.sync.dma_start(
                            xrow[:], x[tb * TB + sub * P: tb * TB + (sub + 1) * P, :])
                        for cc in range(CC):
                            tp = psra.tile([P, P], F32R, tag="tp", bufs=2)
                            nc.tensor.transpose(
                                tp[:], xrow[:, cc * P:(cc + 1) * P], ident[:])
                            nc.vector.tensor_copy(
                                xt[cc][:, sub * P:(sub + 1) * P], tp[:])

                    # router logits for this block: [E, TB]
                    lg_ps = psra.tile([E, TB], F32, tag="lg")
                    for cc in range(CC):
                        nc.tensor.matmul(lg_ps[:], rt[cc][:],
                                         xt[cc][:].bitcast(F32),
                                         start=(cc == 0), stop=(cc == CC - 1))
                    lg_sb = ra2.tile([E, TB], F32, tag="lgsb")
                    nc.vector.tensor_copy(lg_sb[:], lg_ps[:])

                    for sub in range(nsub):
                        j = tb * nsub + sub
                        lgt_ps = psra.tile([P, E], F32, tag="lgt")
                        nc.tensor.transpose(
                            lgt_ps[:], lg_sb[:, sub * P:(sub + 1) * P],
                            ident[:E, :E].bitcast(F32))
                        lgt = ra2.tile([P, E], F32, tag="lgt_sb")
                        nc.vector.tensor_copy(lgt[:], lgt_ps[:])
                        # softmax over the 8 experts (free axis)
                        mx = ra2.tile([P, 1], F32, tag="mx")
                        nc.vector.reduce_max(mx[:], lgt[:],
                                             axis=mybir.AxisListType.X)
                        nmx = ra2.tile([P, 1], F32, tag="nmx")
                        nc.vector.tensor_scalar_mul(nmx[:], mx[:], -1.0)
                        ex = ra2.tile([P, E], F32, tag="ex")
                        nc.scalar.activation(ex[:], lgt[:],
                                             mybir.ActivationFunctionType.Exp,
                                             bias=nmx[:, :1])
                        ssum = ra2.tile([P, 1], F32, tag="ssum")
                        nc.vector.reduce_sum(ssum[:], ex[:],
                                             axis=mybir.AxisListType.X)
                        m1 = ra2.tile([P, 1], F32, tag="m1")
                        nc.vector.reduce_max(m1[:], ex[:],
                                             axis=mybir.AxisListType.X)
                        lt1 = ra2.tile([P, E], F32, tag="lt1")
                        nc.vector.tensor_scalar(lt1[:], ex[:], m1[:, :1], None,
                                                op0=mybir.AluOpType.is_lt)
                        e2 = ra2.tile([P, E], F32, tag="e2")
                        nc.vector.tensor_mul(e2[:], ex[:], lt1[:])
                        m2 = ra2.tile([P, 1], F32, tag="m2")
                        nc.vector.reduce_max(m2[:], e2[:],
                                             axis=mybir.AxisListType.X)
                        ge2 = ra2.tile([P, E], F32, tag="ge2")
                        nc.vector.tensor_scalar(ge2[:], ex[:], m2[:, :1], None,
                                                op0=mybir.AluOpType.is_ge)
                        sel = ra2.tile([P, E], F32, tag="sel")
                        nc.vector.tensor_mul(sel[:], ex[:], ge2[:])
                        selo = ra2.tile([P, E], F32, tag="selo")
                        nc.vector.tensor_mul(selo[:], sel[:], ohb[:])
                        wnum = ra2.tile([P, 1], F32, tag="wnum")
                        nc.vector.reduce_sum(wnum[:], selo[:],
                                             axis=mybir.AxisListType.X)
                        den = ra2.tile([P, 1], F32, tag="den")
                        nc.vector.tensor_add(den[:], m1[:], m2[:])
                        eps = ra2.tile([P, 1], F32, tag="eps")
                        nc.vector.tensor_scalar_mul(eps[:], ssum[:], 1e-8)
                        nc.vector.tensor_add(den[:], den[:], eps[:])
                        rden = ra2.tile([P, 1], F32, tag="rden")
                        nc.vector.reciprocal(rden[:], den[:])
                        nc.vector.tensor_mul(wv[:, j:j + 1], wnum[:], rden[:])

                    # up/gate/act for this block
                    for hc in range(HC):
                        up_ps = psra.tile([P, TB], F32, tag="up", bufs=2)
                        gt_ps = psra.tile([P, TB], F32, tag="gt", bufs=2)
                        for cc in range(CC):
                            nc.tensor.matmul(up_ps[:],
                                             wu[cc][:, hc * P:(hc + 1) * P],
                                             xt[cc][:], start=(cc == 0),
                                             stop=(cc == CC - 1))
                        for cc in range(CC):
                            nc.tensor.matmul(gt_ps[:],
                                             wg[cc][:, hc * P:(hc + 1) * P],
                                             xt[cc][:], start=(cc == 0),
                                             stop=(cc == CC - 1))
                        act_sb = ra2.tile([P, TB], F32R, tag="act")
                        _silu_mul(nc, act_sb[:], up_ps[:], gt_ps[:])
                        nc.sync.dma_start(
                            acts_r[hc * P:(hc + 1) * P, tb * TB:(tb + 1) * TB],
                            act_sb[:])

            # ============ routed expert, phase B: down + scale ============
            with (
                tc.tile_pool(name="rB", bufs=1) as rb,
                tc.tile_pool(name="rB2", bufs=2) as rb2,
                tc.tile_pool(name="psRB", bufs=2, space="PSUM") as psrb,
            ):
                wd = [rb.tile([P, C], F32R, tag=f"rwd{hc}", name=f"rwd{hc}") for hc in range(HC)]
                for hc in range(HC):
                    nc.sync.dma_start(wd[hc][:], rwd[hc * P:(hc + 1) * P, :])
                NG = 4
                tb_per_g = (T // TB) // NG
                g_rows = T // NG          # partial rows per group
                o_rows = g_rows // NCORES  # rs_out rows per group
                for tb in range(T // TB):
                    acts = [rb2.tile([P, TB], F32R, tag=f"ar{hc}", name=f"ar{hc}")
                            for hc in range(HC)]
                    for hc in range(HC):
                        nc.sync.dma_start(
                            acts[hc][:],
                            acts_r[hc * P:(hc + 1) * P, tb * TB:(tb + 1) * TB])
                    for sub in range(TB // P):
                        j = tb * (TB // P) + sub
                        y_sb = rb2.tile([P, C], F32, tag="ysb")
                        for cb in range(C // 512):
                            y_ps = psrb.tile([P, 512], F32, tag="y")
                            for hc in range(HC):
                                nc.tensor.matmul(
                                    y_ps[:],
                                    acts[hc][:, sub * P:(sub + 1) * P],
                                    wd[hc][:, cb * 512:(cb + 1) * 512],
                                    start=(hc == 0), stop=(hc == HC - 1))
                            nc.scalar.activation(
                                y_sb[:, cb * 512:(cb + 1) * 512], y_ps[:],
                                mybir.ActivationFunctionType.Copy,
                                scale=wv[:, j:j + 1])
                        nc.sync.dma_start(partial[j * P:(j + 1) * P, :], y_sb[:])
                    if (tb + 1) % tb_per_g == 0:
                        g = tb // tb_per_g
                        nc.gpsimd.collective_compute(
                            "ReduceScatter",
                            mybir.AluOpType.add,
                            replica_groups=[list(range(NCORES))],
                            ins=[partial[g * g_rows:(g + 1) * g_rows, :]],
                            outs=[rs_out[g * o_rows:(g + 1) * o_rows, :]],
                        )

            # ============ combine: shared add on RS output ============
            with tc.tile_pool(name="fin", bufs=2) as fin:
                for ts in range(TS // P):
                    r_sb = fin.tile([P, C], F32, tag="r")
                    nc.sync.dma_start(r_sb[:], rs_out[ts * P:(ts + 1) * P, :])
                    nc.vector.tensor_add(r_sb[:], r_sb[:], ys[ts][:])
                    nc.sync.dma_start(out[ts * P:(ts + 1) * P, :], r_sb[:])

    nc.compile()
    return nc


_NC_CACHE = None


def kernel(x, shared_Wup, shared_Wgate, shared_Wdown,
           routed_Wup, routed_Wgate, routed_Wdown, router_W):
    global _NC_CACHE
    if _NC_CACHE is None:
        _NC_CACHE = _build_program()
    nc = _NC_CACHE

    xf = np.ascontiguousarray(np.asarray(x, dtype=np.float32).reshape(T, C))
    NG = 4
    g_rows = T // NG
    o_rows = g_rows // NCORES
    in_maps = []
    core_idx = []
    for c in range(NCORES):
        idx = np.concatenate([
            np.arange(g * g_rows + c * o_rows, g * g_rows + (c + 1) * o_rows)
            for g in range(NG)])
        core_idx.append(idx)
    for c in range(NCORES):
        ohv = np.zeros((P, E), np.float32)
        ohv[:, c] = 1.0
        in_maps.append({
            "x": xf,
            "xs": np.ascontiguousarray(xf[core_idx[c], :]),
            "rwu": np.ascontiguousarray(np.asarray(routed_Wup[c], np.float32)),
            "rwg": np.ascontiguousarray(np.asarray(routed_Wgate[c], np.float32)),
            "rwd": np.ascontiguousarray(np.asarray(routed_Wdown[c], np.float32)),
            "swu": np.ascontiguousarray(np.asarray(shared_Wup, np.float32)),
            "swg": np.ascontiguousarray(np.asarray(shared_Wgate, np.float32)),
            "swd": np.ascontiguousarray(np.asarray(shared_Wdown, np.float32)),
            "rtw": np.ascontiguousarray(np.asarray(router_W, np.float32)),
            "ohx": ohv,
            "idn": np.eye(P, dtype=np.float32),
        })

    res = run_bass_kernel_spmd(nc, in_maps, list(range(NCORES)))
    full = np.empty((T, C), np.float32)
    for c in range(NCORES):
        full[core_idx[c]] = res.results[c]["out"]
    return full.reshape(2, 2048, C).astype(np.float32)


if __name__ == "__main__":
    rng = np.random.default_rng(0)
    ins = {
        "x": rng.standard_normal((2, 2048, C), dtype=np.float32),
        "shared_Wup": rng.standard_normal((C, H), dtype=np.float32) * 0.03,
        "shared_Wgate": rng.standard_normal((C, H), dtype=np.float32) * 0.03,
        "shared_Wdown": rng.standard_normal((H, C), dtype=np.float32) * 0.02,
        "routed_Wup": rng.standard_normal((E, C, H), dtype=np.float32) * 0.03,
        "routed_Wgate": rng.standard_normal((E, C, H), dtype=np.float32) * 0.03,
        "routed_Wdown": rng.standard_normal((E, H, C), dtype=np.float32) * 0.02,
        "router_W": rng.standard_normal((C, E), dtype=np.float32) * 0.03,
    }
    out = kernel(**ins)
    print("out", out.shape, out.dtype, float(np.abs(out).mean()))


# revision 12
# speedup vs baseline: 1.0073x; 1.0073x over previous
"""MoE FFN (8 routed experts top-2 + 1 shared expert) on 8 TRN2 NeuronCores.

Sharding: expert-parallel. Core c holds routed expert c's weights and computes
that expert densely for all 4096 tokens, scaled by the (renormalized top-2)
combine weight for expert c (zero for tokens that didn't pick it). Core c also
computes the shared expert for its 512-token slice. A ReduceScatter over the
per-core partials [4096, 1024] sums expert contributions and hands core c the
token slice [512c:512(c+1)); the shared-expert slice is added locally and each
core emits its 512-token output shard, concatenated on the host.

All matmuls run as float32r (full-rate fp32 on the PE when the moving dim is
>= 256), accumulating in fp32 PSUM.
"""

import numpy as np

import concourse.bacc as bacc
import concourse.bass as bass
import concourse.mybir as mybir
import concourse.tile as tile
from concourse.bass_utils import run_bass_kernel_spmd

P = 128
C = 1024          # d_model
H = 2048          # d_expert
T = 4096          # tokens (2*2048)
E = 8             # routed experts = cores
TOPK = 2
TS = T // E       # 512 tokens per core slice
TB = 256          # token block for the routed phases (moving dim >= 256)
CC = C // P       # 8 c-chunks
HC = H // P       # 16 h-chunks
F32 = mybir.dt.float32
F32R = mybir.dt.float32r

NCORES = 8

# Hardware has a native Silu; the simulator doesn't. Flip to False for sim runs.
SILU_NATIVE = True


def _silu_mul(nc, act_sb, up_ps, gt_ps):
    if SILU_NATIVE:
        nc.scalar.activation(act_sb, up_ps,
                             mybir.ActivationFunctionType.Silu)
    else:
        nc.scalar.activation(act_sb, up_ps,
                             mybir.ActivationFunctionType.Sigmoid)
        nc.vector.tensor_mul(act_sb, act_sb, up_ps)
    nc.vector.tensor_mul(act_sb, act_sb, gt_ps)


def _build_program(T=T, C=C, H=H, TS=TS, TB=TB):
    CC = C // P
    HC = H // P
    nc = bacc.Bacc("TRN2", target_bir_lowering=False, debug=False,
                   num_devices=NCORES)

    # ---- per-core inputs ----
    x = nc.dram_tensor("x", [T, C], F32R, kind="ExternalInput")       # replicated
    xs = nc.dram_tensor("xs", [TS, C], F32R, kind="ExternalInput")    # token slice
    rwu = nc.dram_tensor("rwu", [C, H], F32R, kind="ExternalInput")   # expert up
    rwg = nc.dram_tensor("rwg", [C, H], F32R, kind="ExternalInput")   # expert gate
    rwd = nc.dram_tensor("rwd", [H, C], F32R, kind="ExternalInput")   # expert down
    swu = nc.dram_tensor("swu", [C, H], F32R, kind="ExternalInput")
    swg = nc.dram_tensor("swg", [C, H], F32R, kind="ExternalInput")
    swd = nc.dram_tensor("swd", [H, C], F32R, kind="ExternalInput")
    rtw = nc.dram_tensor("rtw", [C, E], F32, kind="ExternalInput")   # router
    ohx = nc.dram_tensor("ohx", [P, E], F32, kind="ExternalInput")    # bcast 1hot
    idn = nc.dram_tensor("idn", [P, P], F32R, kind="ExternalInput")   # identity

    out = nc.dram_tensor("out", [TS, C], F32, kind="ExternalOutput")

    # ---- internal DRAM ----
    acts_s = nc.dram_tensor("acts_s", [H, TS], F32R)       # shared-expert act spill
    acts_r = nc.dram_tensor("acts_r", [H, T], F32R)        # routed act spill
    partial = nc.dram_tensor("partial", [T, C], F32)       # pre-reduce partial
    rs_out = nc.dram_tensor("rs_out", [TS, C], F32)

    with tile.TileContext(nc) as tc:
        with tc.tile_pool(name="persist", bufs=1) as pp:
            ident = pp.tile([P, P], F32R)
            nc.sync.dma_start(ident[:], idn[:])
            ohb = pp.tile([P, E], F32)
            nc.sync.dma_start(ohb[:], ohx[:])

            # per-token combine weight for this core's expert, column j = t-tile
            wv = pp.tile([P, T // P], F32)

            # shared-expert output, kept until after the ReduceScatter
            ys = [pp.tile([P, C], F32, tag=f"ys{i}", name=f"ys{i}") for i in range(TS // P)]

            # ============ routed expert, phase A: router + up/gate/act ==========
            with (
                tc.tile_pool(name="rA", bufs=1) as ra,
                tc.tile_pool(name="rA2", bufs=2) as ra2,
                tc.tile_pool(name="psRA", bufs=1, space="PSUM") as psra,
            ):
                wu = [ra.tile([P, H], F32R, tag=f"rwu{cc}", name=f"rwu{cc}") for cc in range(CC)]
                wg = [ra.tile([P, H], F32R, tag=f"rwg{cc}", name=f"rwg{cc}") for cc in range(CC)]
                rt = [ra.tile([P, E], F32, tag=f"rt{cc}", name=f"rt{cc}") for cc in range(CC)]
                for cc in range(CC):
                    nc.sync.dma_start(wu[cc][:], rwu[cc * P:(cc + 1) * P, :])
                    nc.sync.dma_start(wg[cc][:], rwg[cc * P:(cc + 1) * P, :])
                    nc.sync.dma_start(rt[cc][:], rtw[cc * P:(cc + 1) * P, :])

                for tb in range(T // TB):
                    nsub = TB // P  # 2
                    # load x rows and transpose into xt [C, TB]
                    xt = [ra2.tile([P, TB], F32R, tag=f"xt{cc}", name=f"xt{cc}") for cc in range(CC)]
                    for sub in range(nsub):
                        xrow = ra2.tile([P, C], F32R, tag="xrow")
                        nc.sync.dma_start(
                            xrow[:], x[tb * TB + sub * P: tb * TB + (sub + 1) * P, :])
                        for cc in range(CC):
                            tp = psra.tile([P, P], F32R, tag="tp", bufs=2)
                            nc.tensor.transpose(
                                tp[:], xrow[:, cc * P:(cc + 1) * P], ident[:])
                            nc.vector.tensor_copy(
                                xt[cc][:, sub * P:(sub + 1) * P], tp[:])

                    # router logits for this block: [E, TB]
                    lg_ps = psra.tile([E, TB], F32, tag="lg")
                    for cc in range(CC):
                        nc.tensor.matmul(lg_ps[:], rt[cc][:],
                                         xt[cc][:].bitcast(F32),
                                         start=(cc == 0), stop=(cc == CC - 1))
                    lg_sb = ra2.tile([E, TB], F32, tag="lgsb")
                    nc.vector.tensor_copy(lg_sb[:], lg_ps[:])

                    for sub in range(nsub):
                        j = tb * nsub + sub
                        lgt_ps = psra.tile([P, E], F32, tag="lgt")
                        nc.tensor.transpose(
                            lgt_ps[:], lg_sb[:, sub * P:(sub + 1) * P],
                            ident[:E, :E].bitcast(F32))
                        lgt = ra2.tile([P, E], F32, tag="lgt_sb")
                        nc.vector.tensor_copy(lgt[:], lgt_ps[:])
                        # softmax over the 8 experts (free axis)
                        mx = ra2.tile([P, 1], F32, tag="mx")
                        nc.vector.reduce_max(mx[:], lgt[:],
                                             axis=mybir.AxisListType.X)
                        nmx = ra2.tile([P, 1], F32, tag="nmx")
                        nc.vector.tensor_scalar_mul(nmx[:], mx[:], -1.0)
                        ex = ra2.tile([P, E], F32, tag="ex")
                        nc.scalar.activation(ex[:], lgt[:],
                                             mybir.ActivationFunctionType.Exp,
                                             bias=nmx[:, :1])
                        ssum = ra2.tile([P, 1], F32, tag="ssum")
                        nc.vector.reduce_sum(ssum[:], ex[:],
                                             axis=mybir.AxisListType.X)
                        m1 = ra2.tile([P, 1], F32, tag="m1")
                        nc.vector.reduce_max(m1[:], ex[:],
                                             axis=mybir.AxisListType.X)
                        lt1 = ra2.tile([P, E], F32, tag="lt1")
                        nc.vector.tensor_scalar(lt1[:], ex[:], m1[:, :1], None,
                                                op0=mybir.AluOpType.is_lt)
                        e2 = ra2.tile([P, E], F32, tag="e2")
                        nc.vector.tensor_mul(e2[:], ex[:], lt1[:])
                        m2 = ra2.tile([P, 1], F32, tag="m2")
                        nc.vector.reduce_max(m2[:], e2[:],
                                             axis=mybir.AxisListType.X)
                        ge2 = ra2.tile([P, E], F32, tag="ge2")
                        nc.vector.tensor_scalar(ge2[:], ex[:], m2[:, :1], None,
                                                op0=mybir.AluOpType.is_ge)
                        sel = ra2.tile([P, E], F32, tag="sel")
                        nc.vector.tensor_mul(sel[:], ex[:], ge2[:])
                        selo = ra2.tile([P, E], F32, tag="selo")
                        nc.vector.tensor_mul(selo[:], sel[:], ohb[:])
                        wnum = ra2.tile([P, 1], F32, tag="wnum")
                        nc.vector.reduce_sum(wnum[:], selo[:],
                                             axis=mybir.AxisListType.X)
                        den = ra2.tile([P, 1], F32, tag="den")
                        nc.vector.tensor_add(den[:], m1[:], m2[:])
                        eps = ra2.tile([P, 1], F32, tag="eps")
                        nc.vector.tensor_scalar_mul(eps[:], ssum[:], 1e-8)
                        nc.vector.tensor_add(den[:], den[:], eps[:])
                        rden = ra2.tile([P, 1], F32, tag="rden")
                        nc.vector.reciprocal(rden[:], den[:])
                        nc.vector.tensor_mul(wv[:, j:j + 1], wnum[:], rden[:])

                    # up/gate/act for this block
                    for hc in range(HC):
                        up_ps = psra.tile([P, TB], F32, tag="up", bufs=2)
                        gt_ps = psra.tile([P, TB], F32, tag="gt", bufs=2)
                        for cc in range(CC):
                            nc.tensor.matmul(up_ps[:],
                                             wu[cc][:, hc * P:(hc + 1) * P],
                                             xt[cc][:], start=(cc == 0),
                                             stop=(cc == CC - 1))
                        for cc in range(CC):
                            nc.tensor.matmul(gt_ps[:],
                                             wg[cc][:, hc * P:(hc + 1) * P],
                                             xt[cc][:], start=(cc == 0),
                                             stop=(cc == CC - 1))
                        act_sb = ra2.tile([P, TB], F32R, tag="act")
                        _silu_mul(nc, act_sb[:], up_ps[:], gt_ps[:])
                        nc.sync.dma_start(
                            acts_r[hc * P:(hc + 1) * P, tb * TB:(tb + 1) * TB],
                            act_sb[:])

            # ============ routed expert, phase B: down + scale ============
            with (
                tc.tile_pool(name="rB", bufs=1) as rb,
                tc.tile_pool(name="rB2", bufs=2) as rb2,
                tc.tile_pool(name="psRB", bufs=2, space="PSUM") as psrb,
            ):
                wd = [rb.tile([P, C], F32R, tag=f"rwd{hc}", name=f"rwd{hc}") for hc in range(HC)]
                for hc in range(HC):
                    nc.sync.dma_start(wd[hc][:], rwd[hc * P:(hc + 1) * P, :])
                NG = 4
                tb_per_g = (T // TB) // NG
                g_rows = T // NG          # partial rows per group
                o_rows = g_rows // NCORES  # rs_out rows per group
                for tb in range(T // TB):
                    acts = [rb2.tile([P, TB], F32R, tag=f"ar{hc}", name=f"ar{hc}")
                            for hc in range(HC)]
                    for hc in range(HC):
                        nc.sync.dma_start(
                            acts[hc][:],
                            acts_r[hc * P:(hc + 1) * P, tb * TB:(tb + 1) * TB])
                    for sub in range(TB // P):
                        j = tb * (TB // P) + sub
                        y_sb = rb2.tile([P, C], F32, tag="ysb")
                        for cb in range(C // 512):
                            y_ps = psrb.tile([P, 512], F32, tag="y")
                            for hc in range(HC):
                                nc.tensor.matmul(
                                    y_ps[:],
                                    acts[hc][:, sub * P:(sub + 1) * P],
                                    wd[hc][:, cb * 512:(cb + 1) * 512],
                                    start=(hc == 0), stop=(hc == HC - 1))
                            nc.scalar.activation(
                                y_sb[:, cb * 512:(cb + 1) * 512], y_ps[:],
                                mybir.ActivationFunctionType.Copy,
                                scale=wv[:, j:j + 1])
                        nc.sync.dma_start(partial[j * P:(j + 1) * P, :], y_sb[:])
                    if (tb + 1) % tb_per_g == 0:
                        g = tb // tb_per_g
                        nc.gpsimd.collective_compute(
                            "ReduceScatter",
                            mybir.AluOpType.add,
                            replica_groups=[list(range(NCORES))],
                            ins=[partial[g * g_rows:(g + 1) * g_rows, :]],
                            outs=[rs_out[g * o_rows:(g + 1) * o_rows, :]],
                        )

            # ============ shared expert, phase A: up/gate/act ============
            with (
                tc.tile_pool(name="sA", bufs=1) as sa,
                tc.tile_pool(name="sA2", bufs=2) as sa2,
                tc.tile_pool(name="psA", bufs=2, space="PSUM") as psa,
            ):
                # transpose xs -> xst [C, TS] in SBUF
                xst = [sa.tile([P, TS], F32R, tag=f"xst{cc}", name=f"xst{cc}") for cc in range(CC)]
                for ts in range(TS // P):
                    xrow = sa2.tile([P, C], F32R, tag="xrow")
                    nc.sync.dma_start(xrow[:], xs[ts * P:(ts + 1) * P, :])
                    for cc in range(CC):
                        tp = psa.tile([P, P], F32R, tag="tp")
                        nc.tensor.transpose(tp[:], xrow[:, cc * P:(cc + 1) * P],
                                            ident[:])
                        nc.vector.tensor_copy(xst[cc][:, ts * P:(ts + 1) * P], tp[:])

                wu = [sa.tile([P, H], F32R, tag=f"swu{cc}", name=f"swu{cc}") for cc in range(CC)]
                wg = [sa.tile([P, H], F32R, tag=f"swg{cc}", name=f"swg{cc}") for cc in range(CC)]
                for cc in range(CC):
                    nc.sync.dma_start(wu[cc][:], swu[cc * P:(cc + 1) * P, :])
                    nc.sync.dma_start(wg[cc][:], swg[cc * P:(cc + 1) * P, :])

                for hc in range(HC):
                    up_ps = psa.tile([P, TS], F32, tag="up")
                    gt_ps = psa.tile([P, TS], F32, tag="gt")
                    for cc in range(CC):
                        nc.tensor.matmul(up_ps[:], wu[cc][:, hc * P:(hc + 1) * P],
                                         xst[cc][:], start=(cc == 0),
                                         stop=(cc == CC - 1))
                    for cc in range(CC):
                        nc.tensor.matmul(gt_ps[:], wg[cc][:, hc * P:(hc + 1) * P],
                                         xst[cc][:], start=(cc == 0),
                                         stop=(cc == CC - 1))
                    act_sb = sa2.tile([P, TS], F32R, tag="act")
                    _silu_mul(nc, act_sb[:], up_ps[:], gt_ps[:])
                    nc.sync.dma_start(acts_s[hc * P:(hc + 1) * P, :], act_sb[:])

            # ============ shared expert, phase B: down ============
            with (
                tc.tile_pool(name="sB", bufs=1) as sb,
                tc.tile_pool(name="psB", bufs=2, space="PSUM") as psb,
            ):
                wd = [sb.tile([P, C], F32R, tag=f"swd{hc}", name=f"swd{hc}") for hc in range(HC)]
                for hc in range(HC):
                    nc.sync.dma_start(wd[hc][:], swd[hc * P:(hc + 1) * P, :])
                acts = [sb.tile([P, TS], F32R, tag=f"as{hc}", name=f"as{hc}") for hc in range(HC)]
                for hc in range(HC):
                    nc.sync.dma_start(acts[hc][:], acts_s[hc * P:(hc + 1) * P, :])
                for ts in range(TS // P):
                    for cb in range(C // 512):
                        y_ps = psb.tile([P, 512], F32, tag="y")
                        for hc in range(HC):
                            nc.tensor.matmul(
                                y_ps[:],
                                acts[hc][:, ts * P:(ts + 1) * P],
                                wd[hc][:, cb * 512:(cb + 1) * 512],
                                start=(hc == 0), stop=(hc == HC - 1))
                        nc.vector.tensor_copy(ys[ts][:, cb * 512:(cb + 1) * 512],
                                              y_ps[:])

            # ============ combine: shared add on RS output ============
            with tc.tile_pool(name="fin", bufs=2) as fin:
                for ts in range(TS // P):
                    r_sb = fin.tile([P, C], F32, tag="r")
                    nc.sync.dma_start(r_sb[:], rs_out[ts * P:(ts + 1) * P, :])
                    nc.vector.tensor_add(r_sb[:], r_sb[:], ys[ts][:])
                    nc.sync.dma_start(out[ts * P:(ts + 1) * P, :], r_sb[:])

    nc.compile()
    return nc


_NC_CACHE = None


def kernel(x, shared_Wup, shared_Wgate, shared_Wdown,
           routed_Wup, routed_Wgate, routed_Wdown, router_W):
    global _NC_CACHE
    if _NC_CACHE is None:
        _NC_CACHE = _build_program()
    nc = _NC_CACHE

    xf = np.ascontiguousarray(np.asarray(x, dtype=np.float32).reshape(T, C))
    NG = 4
    g_rows = T // NG
    o_rows = g_rows // NCORES
    in_maps = []
    core_idx = []
    for c in range(NCORES):
        idx = np.concatenate([
            np.arange(g * g_rows + c * o_rows, g * g_rows + (c + 1) * o_rows)
            for g in range(NG)])
        core_idx.append(idx)
    for c in range(NCORES):
        ohv = np.zeros((P, E), np.float32)
        ohv[:, c] = 1.0
        in_maps.append({
            "x": xf,
            "xs": np.ascontiguousarray(xf[core_idx[c], :]),
            "rwu": np.ascontiguousarray(np.asarray(routed_Wup[c], np.float32)),
            "rwg": np.ascontiguousarray(np.asarray(routed_Wgate[c], np.float32)),
            "rwd": np.ascontiguousarray(np.asarray(routed_Wdown[c], np.float32)),
            "swu": np.ascontiguousarray(np.asarray(shared_Wup, np.float32)),
            "swg": np.ascontiguousarray(np.asarray(shared_Wgate, np.float32)),
            "swd": np.ascontiguousarray(np.asarray(shared_Wdown, np.float32)),
            "rtw": np.ascontiguousarray(np.asarray(router_W, np.float32)),
            "ohx": ohv,
            "idn": np.eye(P, dtype=np.float32),
        })

    res = run_bass_kernel_spmd(nc, in_maps, list(range(NCORES)))
    full = np.empty((T, C), np.float32)
    for c in range(NCORES):
        full[core_idx[c]] = res.results[c]["out"]
    return full.reshape(2, 2048, C).astype(np.float32)


if __name__ == "__main__":
    rng = np.random.default_rng(0)
    ins = {
        "x": rng.standard_normal((2, 2048, C), dtype=np.float32),
        "shared_Wup": rng.standard_normal((C, H), dtype=np.float32) * 0.03,
        "shared_Wgate": rng.standard_normal((C, H), dtype=np.float32) * 0.03,
        "shared_Wdown": rng.standard_normal((H, C), dtype=np.float32) * 0.02,
        "routed_Wup": rng.standard_normal((E, C, H), dtype=np.float32) * 0.03,
        "routed_Wgate": rng.standard_normal((E, C, H), dtype=np.float32) * 0.03,
        "routed_Wdown": rng.standard_normal((E, H, C), dtype=np.float32) * 0.02,
        "router_W": rng.standard_normal((C, E), dtype=np.float32) * 0.03,
    }
    out = kernel(**ins)
    print("out", out.shape, out.dtype, float(np.abs(out).mean()))
